# revision 16
# baseline (speedup 1.0000x reference)
"""CondConv2d (MoE routed conv) Trainium2 kernel.

Math: out[b] = sum_e routing[b,e] * conv3x3(x[b], W[e])
Since the expert mix is linear in W, this equals
    out[b] = conv3x3(x[b], Wmix_b),  Wmix_b = sum_e routing[b,e] * W[e]
which needs 1 conv per sample instead of E=4 (4x less PE work).

Sharding: data-parallel over batch, B=16 -> 2 samples per core on 8 cores.
Weights (all 4 experts, transposed to [ci, tap, e, co] on host) are
replicated; the per-sample mix happens on-device on the Vector engine.

Conv as implicit GEMM: x is zero-padded on host to [ci, 58, 58]; for each
of 9 taps the matmul streams a shifted window of the padded image
(rhs = xpad[:, blk*8+kh : +8, kw : kw+56], N=448) against the tap's mixed
weight slice (lhsT = Wmix[ci, co], K=ci on partitions), accumulating all
9 taps into one PSUM bank (fp32). 7 row-blocks of 8 rows cover the 56
output rows.

Numerics: x and W are fp16 on the wire; matmuls run fp16 at 1 cycle/row
with fp32 PSUM accumulation (~4e-4 L2 rel err). The output is stored as
fp16 (upcast to fp32 on host), halving store traffic; the extra fp16
rounding adds ~3e-4, total ~5e-4 -- far inside the 2e-2 gate.

Schedule: the whole kernel is DMA-need-ordered. Loads are issued in strict
global need order, alternating between the sync and scalar DGE rings so
both rings' FIFOs drain in need order while sharing the ~330 GB/s HBM
bandwidth: wt tap0+routing first (gates the first weight mix), then the
four x0 row-chunks (tap-outer sample 0 sweeps the full image on its first
tap), with the per-tap weight chunks interleaved where they are needed,
then sample 1's x. Sample 0 runs tap-outer over 7 live PSUM banks;
sample 1 runs block-outer (9 taps into one bank, then drain) so its
output streams out incrementally. PSUM drains are split between the
Scalar (activation-copy) and Vector engines and convert fp32->fp16;
stores are batched into few large-line DMAs. Dummy matmuls on a zeroed
tile cover the initial load phase to keep the PE HAM clock-gate warm.
"""

import os
import sys

os.environ.setdefault("MYCRO_LOCAL_CACHE", "1")
for _p in ("/opt/trn_rl_repo",):
    if _p not in sys.path:
        sys.path.insert(0, _p)

import numpy as np

B, CIN, COUT, H, W_SP = 16, 128, 128, 56, 56
E, KH, KW = 4, 3, 3
NCORES = 8
SPC = B // NCORES          # samples per core
HP, WP = H + 2, W_SP + 2   # padded spatial
NTAP = KH * KW
RPB = 8                    # output rows per matmul block
NBLK = H // RPB
NT = RPB * W_SP            # moving-operand free size per matmul (448)
N_WARM = 7                 # big (N=512) HAM warm-up dummy matmuls
N_WARM_SM = 8              # small (N=128) bridge dummies before the stream
# The PE HAM clock needs ~4us of GAPLESS activity to ramp to 2.4 GHz and
# any idle resets it; the x/weight chase in sample-0's first taps has
# unavoidable DMA waits, so small dummy matmuls (N=128) pad those gaps.
# Keys are (tap, blk) after whose matmul the pads are emitted; sized to
# overshoot the expected wait (overshoot costs ~0.1us/dummy, a reset
# costs ~2us of half-rate stream).
# keyed by (pass_index, blk): pass 0/1 are the tap-0 half-mixes
PADS = {(0, 3): 3, (0, 5): 2, (1, 6): 3, (2, 6): 3, (3, 6): 2}

# sample-0 x row chunks (start_row, n_rows) and block->chunk map; chunks
# overlap by 2 rows so each 8-row output block reads one chunk only. The
# first two chunks are small so the stream starts as soon as possible.
XCH0 = [(0, 10), (8, 10), (16, 18), (32, 18), (48, 10)]
BLK_CH0 = [0, 1, 2, 2, 3, 3, 4]
XCH1 = [(0, 34), (32, 26)]
# sample-1 row blocks (r0, nr, chunk); small final block shortens the tail
BLKS1 = [(0, 8, 0), (8, 8, 0), (16, 8, 0), (24, 8, 0),
         (32, 8, 1), (40, 8, 1), (48, 6, 1), (54, 2, 1)]
# weight-tap DMA chunks (start_tap, n_taps) on the GpSimd prefetch ring
WTCH = [(1, 4), (5, 4)]
# sample-0 mix chunks for taps 1-8 (tap 0 is handled as two 2-expert
# half-mixes for latency); per-tap early, wider later. Each chunk gets
# its OWN tile (matmul weight reads are tracked whole-tile, so chunks
# sharing a tile would serialize behind earlier matmuls)
MIXCH = [(1, 1), (2, 1), (3, 2), (5, 2), (7, 2)]
# sample-1 mix chunks (vector runs them after sample 0's, well before use)
MIXCH1 = [(0, 3), (3, 3), (6, 3)]

_cached_nc = None


def _build_nc():
    import concourse.tile as tile
    from concourse import bacc, mybir

    f32 = mybir.dt.float32
    f16 = mybir.dt.float16
    MUL, ADD = mybir.AluOpType.mult, mybir.AluOpType.add

    nc = bacc.Bacc(
        "TRN2", target_bir_lowering=False, debug=False, num_devices=NCORES
    )

    xpad_d = nc.dram_tensor(
        "xpad", [SPC, CIN, HP * WP], f16, kind="ExternalInput"
    ).ap()
    # host layout: [ci, (rb | tap, e, co)] — routing scalars (fp32 bits
    # packed into 2 fp16 slots each; tensor_scalar wants fp32 scalars)
    # share the weight tensor so one DMA delivers both rb and tap 0
    TAPW = E * COUT          # 512 halfs per tap in wt
    RBW = SPC * E * 2        # fp32 scalars as fp16 slot pairs
    wt_d = nc.dram_tensor(
        "wt", [CIN, RBW + NTAP * TAPW], f16, kind="ExternalInput"
    ).ap()
    out_d = nc.dram_tensor(
        "out", [SPC, COUT, H * W_SP], f16, kind="ExternalOutput"
    ).ap()

    with tile.TileContext(nc) as tc:
        with (
            tc.tile_pool(name="const", bufs=1) as cst,
            tc.tile_pool(name="x", bufs=1) as xpool,
            tc.tile_pool(name="wmix", bufs=1) as wmp,
            tc.tile_pool(name="ob", bufs=2) as opool,
            tc.tile_pool(name="ps", bufs=8, space="PSUM") as pspool,
        ):
            # --- HAM warm-up: dummy matmuls on a zeroed tile during loads
            zt = cst.tile([128, 512], f16, tag="zero")
            nc.gpsimd.memset(zt[:], 0.0)
            warm_ps = pspool.tile([128, 512], f32, tag="ps")
            for _ in range(N_WARM):
                nc.tensor.matmul(
                    warm_ps[:], zt[:, :128], zt[:], start=True, stop=True
                )
            for _ in range(N_WARM_SM):
                nc.tensor.matmul(
                    warm_ps[:, :128], zt[:, :128], zt[:, :128],
                    start=True, stop=True,
                )

            wt_t = cst.tile([CIN, RBW + NTAP * TAPW], f16, tag="wt")
            rb_t = wt_t[:, 0:RBW].bitcast(f32)  # [128, SPC*E] fp32

            def load_wt_chunk(t0, ntaps, eng):
                # first chunk also carries the routing scalars
                lo = 0 if t0 == 0 else RBW + t0 * TAPW
                sl = slice(lo, RBW + (t0 + ntaps) * TAPW)
                eng.dma_start(wt_t[:, sl], wt_d[:, sl])

            def load_x_chunk(s, xtiles, xch, c, eng):
                r0, nr = xch[c]
                xt = xpool.tile([CIN, nr * WP], f16, tag=f"x{s}_{c}",
                                name=f"x{s}_{c}")
                sl = slice(r0 * WP, (r0 + nr) * WP)
                eng.dma_start(xt[:], xpad_d[s][:, sl])
                xtiles[c] = xt

            # Three DGE rings, all need-ordered. Sync and scalar split the
            # x chunks (each ring's first item is one of the two pieces
            # that gate the first matmul: tap-0 weights / x rows 0-9);
            # the idle GpSimd ring prefetches all remaining weight taps
            # so the per-tap mixes never wait behind x traffic. Stores
            # later reuse the sync ring.
            x0t = [None] * len(XCH0)
            x1t = [None] * len(XCH1)
            load_wt_chunk(0, 1, nc.sync)           # rb + tap 0 weights
            load_x_chunk(0, x0t, XCH0, 0, nc.scalar)
            load_x_chunk(0, x0t, XCH0, 1, nc.sync)
            load_x_chunk(0, x0t, XCH0, 2, nc.scalar)
            load_x_chunk(0, x0t, XCH0, 3, nc.sync)
            load_x_chunk(0, x0t, XCH0, 4, nc.scalar)
            for t0, ntaps in WTCH:
                load_wt_chunk(t0, ntaps, nc.gpsimd)
            load_x_chunk(1, x1t, XCH1, 0, nc.sync)
            load_x_chunk(1, x1t, XCH1, 1, nc.scalar)

            # warm the Activation engine's Copy table during the load
            # phase so the first real PSUM drain doesn't pay the load
            aw = cst.tile([128, 1], f16, tag="actwarm")
            nc.scalar.copy(aw[:], zt[:, 0:1])

            wt3 = wt_t[:, RBW:].rearrange("p (t e c) -> p t e c", t=NTAP, e=E)

            def mix(dst3, s, t0, t1, e_lo=0, e_hi=E):
                """dst3 = sum_{e in [e_lo,e_hi)} rb[s,e] * wt[:, t0:t1, e, :]"""
                first = True
                for e in range(e_lo, e_hi):
                    sc = rb_t[:, s * E + e : s * E + e + 1]
                    src = wt3[:, t0:t1, e, :]
                    if first:
                        nc.vector.tensor_scalar_mul(dst3, src, sc)
                        first = False
                    else:
                        nc.vector.scalar_tensor_tensor(
                            dst3, src, sc, dst3, MUL, ADD
                        )

            # tap 0 as two 2-expert half-mixes: the first matmul only
            # waits a 2-op mix chain instead of 4, and the second half
            # rides the PSUM accumulation as an extra matmul pass (free
            # during the x-load chase)
            wm0a = wmp.tile([CIN, COUT], f16, tag="wm0a")
            wm0b = wmp.tile([CIN, COUT], f16, tag="wm0b")
            mix(wm0a.rearrange("p (t c) -> p t c", t=1), 0, 0, 1, 0, 2)
            mix(wm0b.rearrange("p (t c) -> p t c", t=1), 0, 0, 1, 2, 4)

            def mix_chunks(s, chlist, prefix):
                out = {}
                for c, (t0, ntc) in enumerate(chlist):
                    wmt = wmp.tile(
                        [CIN, ntc * COUT], f16, tag=f"{prefix}{c}",
                        name=f"{prefix}{c}",
                    )
                    wm3 = wmt.rearrange("p (t c) -> p t c", t=ntc)
                    mix(wm3, s, t0, t0 + ntc)
                    for tt in range(t0, t0 + ntc):
                        out[tt] = (wmt, tt - t0)
                return out

            wm0 = mix_chunks(0, MIXCH, "wm0_")
            wm1 = mix_chunks(1, MIXCH1, "wm1_")

            def rhs_ap(xtiles, c, r0, nr, kh, kw):
                xch = XCH0 if xtiles is x0t else XCH1
                loc = r0 - xch[c][0]
                x3 = xtiles[c][:].rearrange("p (h w) -> p h w", w=WP)
                return x3[:, loc + kh : loc + kh + nr, kw : kw + W_SP]

            def copy_block(eng, ob, ps, r0, nr):
                sl = slice(r0 * W_SP, (r0 + nr) * W_SP)
                if eng is nc.scalar:
                    nc.scalar.copy(ob[:, sl], ps[:])
                else:
                    eng.tensor_copy(ob[:, sl], ps[:])

            # ---- sample 0: tap-outer over 7 live PSUM banks
            ps_map = {}
            for blk in range(NBLK):
                ps_map[blk] = pspool.tile(
                    [COUT, NT], f32, tag="ps", name=f"ps0_{blk}"
                )

            def pad_pe(n):
                for _ in range(n):
                    nc.tensor.matmul(
                        warm_ps[:, :128], zt[:, :128], zt[:, :128],
                        start=True, stop=True,
                    )

            passes = [(0, wm0a, 0, True, False), (0, wm0b, 0, False, False)]
            for t in range(1, NTAP):
                chunk, loc = wm0[t]
                passes.append((t, chunk, loc, False, t == NTAP - 1))

            for pi, (t, chunk, loc, start, stop) in enumerate(passes):
                kh, kw = divmod(t, KW)
                for blk in range(NBLK):
                    nc.tensor.matmul(
                        ps_map[blk][:],
                        chunk[:, loc * COUT : (loc + 1) * COUT],
                        rhs_ap(x0t, BLK_CH0[blk], blk * RPB, RPB, kh, kw),
                        start=start,
                        stop=stop,
                        skip_group_check=True,
                    )
                    pad_pe(PADS.get((pi, blk), 0))

            # drain sample 0: fp32 PSUM -> fp16 SBUF on Scalar/Vector in
            # parallel, then one large-line store for the whole sample
            ob0 = opool.tile([COUT, H * W_SP], f16, tag="ob")
            for blk in range(NBLK):
                eng = nc.scalar if blk % 2 == 0 else nc.vector
                copy_block(eng, ob0, ps_map[blk], blk * RPB, RPB)
            nc.sync.dma_start(out_d[0], ob0[:])

            # ---- sample 1: block-outer, drains incrementally with
            # batched stores (blocks 0-3, 4-5, 6, 7)
            ob1 = opool.tile([COUT, H * W_SP], f16, tag="ob")
            store_after = {3: slice(0, 32 * W_SP),
                           5: slice(32 * W_SP, 48 * W_SP),
                           6: slice(48 * W_SP, 54 * W_SP),
                           7: slice(54 * W_SP, 56 * W_SP)}
            for blk, (r0, nr, c) in enumerate(BLKS1):
                ps = pspool.tile(
                    [COUT, nr * W_SP], f32, tag="ps", name=f"ps1_{blk}"
                )
                for t in range(NTAP):
                    kh, kw = divmod(t, KW)
                    chunk, loc = wm1[t]
                    nc.tensor.matmul(
                        ps[:],
                        chunk[:, loc * COUT : (loc + 1) * COUT],
                        rhs_ap(x1t, c, r0, nr, kh, kw),
                        start=(t == 0),
                        stop=(t == NTAP - 1),
                    )
                # last block's copy AND store both on scalar: same-engine
                # ordering avoids a cross-engine semaphore hop in the tail
                last = blk == len(BLKS1) - 1
                eng = nc.scalar if (blk % 2 == 0 or last) else nc.vector
                copy_block(eng, ob1, ps, r0, nr)
                if blk in store_after:
                    # last store goes out on the idle Scalar ring so its
                    # descriptor generation isn't queued behind the
                    # previous store on sync (shorter kernel tail)
                    sl = store_after[blk]
                    seng = nc.scalar if blk == len(BLKS1) - 1 else nc.sync
                    seng.dma_start(out_d[1][:, sl], ob1[:, sl])

    nc.compile()
    return nc


def _get_nc():
    global _cached_nc
    if _cached_nc is None:
        _cached_nc = _build_nc()
    return _cached_nc


def _prep_inputs(x, routing_weights, W):
    x = np.ascontiguousarray(x, dtype=np.float32)
    routing_weights = np.ascontiguousarray(routing_weights, dtype=np.float32)
    W = np.ascontiguousarray(W, dtype=np.float32)

    xpad = np.zeros((B, CIN, HP, WP), np.float16)
    xpad[:, :, 1 : H + 1, 1 : W_SP + 1] = x.reshape(B, CIN, H, W_SP)
    xpad = xpad.reshape(B, CIN, HP * WP)

    # W[e, co, ci, kh, kw] -> wt[ci, (kh, kw, e, co)], with the per-core
    # routing scalars (broadcast over partitions) prepended
    wt = np.ascontiguousarray(
        np.transpose(W, (2, 3, 4, 0, 1)).astype(np.float16)
    ).reshape(CIN, NTAP * E * COUT)

    in_maps = []
    for c in range(NCORES):
        r = routing_weights[c * SPC : (c + 1) * SPC]  # fp32 [SPC, E]
        rb16 = r.reshape(1, SPC * E).view(np.float16)  # fp32 bits as fp16 pairs
        rb = np.broadcast_to(rb16, (128, SPC * E * 2))
        in_maps.append(
            {
                "xpad": xpad[c * SPC : (c + 1) * SPC],
                "wt": np.ascontiguousarray(np.concatenate([rb, wt], axis=1)),
            }
        )
    return in_maps


def _run(in_maps, **kwargs):
    from concourse import bass_utils

    nc = _get_nc()
    res = bass_utils.run_bass_kernel_spmd(
        nc, in_maps, core_ids=list(range(NCORES)), **kwargs
    )
    out = np.concatenate(
        [res.results[c]["out"].astype(np.float32) for c in range(NCORES)],
        axis=0,
    ).reshape(B, COUT, H, W_SP)
    return out, res


def kernel(x, routing_weights, W):
    in_maps = _prep_inputs(x, routing_weights, W)
    out, _ = _run(in_maps)
    return out


# revision 19
# speedup vs baseline: 1.0988x; 1.0988x over previous
"""CondConv2d (MoE routed conv) Trainium2 kernel.

Math: out[b] = sum_e routing[b,e] * conv3x3(x[b], W[e])
Since the expert mix is linear in W, this equals
    out[b] = conv3x3(x[b], Wmix_b),  Wmix_b = sum_e routing[b,e] * W[e]
which needs 1 conv per sample instead of E=4 (4x less PE work).

Sharding: data-parallel over batch, B=16 -> 2 samples per core on 8 cores.
Weights (all 4 experts, transposed to [ci, tap, e, co] on host) are
replicated; the per-sample mix happens on-device on the Vector engine.

Conv as implicit GEMM: x is zero-padded on host to [ci, 58, 58]; for each
of 9 taps the matmul streams a shifted window of the padded image
(rhs = xpad[:, blk*8+kh : +8, kw : kw+56], N=448) against the tap's mixed
weight slice (lhsT = Wmix[ci, co], K=ci on partitions), accumulating all
9 taps into one PSUM bank (fp32). 7 row-blocks of 8 rows cover the 56
output rows.

Numerics: x and W are fp16 on the wire; matmuls run fp16 at 1 cycle/row
with fp32 PSUM accumulation (~4e-4 L2 rel err). The output is stored as
fp16 (upcast to fp32 on host), halving store traffic; the extra fp16
rounding adds ~3e-4, total ~5e-4 -- far inside the 2e-2 gate.

Schedule: the whole kernel is DMA-need-ordered. Loads are issued in strict
global need order, alternating between the sync and scalar DGE rings so
both rings' FIFOs drain in need order while sharing the ~330 GB/s HBM
bandwidth: wt tap0+routing first (gates the first weight mix), then the
four x0 row-chunks (tap-outer sample 0 sweeps the full image on its first
tap), with the per-tap weight chunks interleaved where they are needed,
then sample 1's x. Sample 0 runs tap-outer over 7 live PSUM banks;
sample 1 runs block-outer (9 taps into one bank, then drain) so its
output streams out incrementally. PSUM drains are split between the
Scalar (activation-copy) and Vector engines and convert fp32->fp16;
stores are batched into few large-line DMAs. Dummy matmuls on a zeroed
tile cover the initial load phase to keep the PE HAM clock-gate warm.
"""

import os
import sys

os.environ.setdefault("MYCRO_LOCAL_CACHE", "1")
for _p in ("/opt/trn_rl_repo",):
    if _p not in sys.path:
        sys.path.insert(0, _p)

import numpy as np

B, CIN, COUT, H, W_SP = 16, 128, 128, 56, 56
E, KH, KW = 4, 3, 3
NCORES = 8
SPC = B // NCORES          # samples per core
HP, WP = H + 2, W_SP + 2   # padded spatial
NTAP = KH * KW
RPB = 8                    # output rows per matmul block
NBLK = H // RPB
NT = RPB * W_SP            # moving-operand free size per matmul (448)
N_WARM = 7                 # big (N=512) HAM warm-up dummy matmuls
N_WARM_SM = 8              # small (N=128) bridge dummies before the stream
# The PE HAM clock needs ~4us of GAPLESS activity to ramp to 2.4 GHz and
# any idle resets it; the x/weight chase in sample-0's first taps has
# unavoidable DMA waits, so small dummy matmuls (N=128) pad those gaps.
# Keys are (tap, blk) after whose matmul the pads are emitted; sized to
# overshoot the expected wait (overshoot costs ~0.1us/dummy, a reset
# costs ~2us of half-rate stream).
# keyed by (pass_index, blk): pass 0/1 are the tap-0 half-mixes
PADS = {(0, 1): 8, (0, 3): 4, (0, 5): 2, (0, 6): 2, (1, 6): 2, (2, 6): 2}

# sample-0 x row chunks (start_row, n_rows) and block->chunk map; chunks
# overlap by 2 rows so each 8-row output block reads one chunk only. The
# first two chunks are small so the stream starts as soon as possible.
XCH0 = [(0, 10), (8, 10), (16, 18), (32, 18), (48, 10)]
BLK_CH0 = [0, 1, 2, 2, 3, 3, 4]
XCH1 = [(0, 34), (32, 26)]
# sample-1 row blocks (r0, nr, chunk); small final block shortens the tail
BLKS1 = [(0, 8, 0), (8, 8, 0), (16, 8, 0), (24, 8, 0),
         (32, 8, 1), (40, 8, 1), (48, 6, 1), (54, 2, 1)]
# weight-tap DMA chunks (start_tap, n_taps): small chunks early so their
# completion semaphores fire as soon as the bytes land
WTCH = [(1, 1), (2, 1), (3, 1), (4, 2), (6, 3)]
# sample-0 mix chunks for taps 1-8 (tap 0 is handled as two 2-expert
# half-mixes for latency); per-tap early, wider later. Each chunk gets
# its OWN tile (matmul weight reads are tracked whole-tile, so chunks
# sharing a tile would serialize behind earlier matmuls)
MIXCH = [(1, 1), (2, 1), (3, 2), (5, 2), (7, 2)]
# sample-1 mix chunks (vector runs them after sample 0's, well before use)
MIXCH1 = [(0, 3), (3, 3), (6, 3)]

_cached_nc = None


def _build_nc():
    import concourse.tile as tile
    from concourse import bacc, mybir

    f32 = mybir.dt.float32
    f16 = mybir.dt.float16
    MUL, ADD = mybir.AluOpType.mult, mybir.AluOpType.add

    nc = bacc.Bacc(
        "TRN2", target_bir_lowering=False, debug=False, num_devices=NCORES
    )

    xpad_d = nc.dram_tensor(
        "xpad", [SPC, CIN, HP * WP], f16, kind="ExternalInput"
    ).ap()
    # host layout: [ci, (rb | tap, e, co)] — routing scalars (fp32 bits
    # packed into 2 fp16 slots each; tensor_scalar wants fp32 scalars)
    # share the weight tensor so one DMA delivers both rb and tap 0
    TAPW = E * COUT          # 512 halfs per tap in wt
    RBW = SPC * E * 2        # fp32 scalars as fp16 slot pairs
    wt_d = nc.dram_tensor(
        "wt", [CIN, RBW + NTAP * TAPW], f16, kind="ExternalInput"
    ).ap()
    out_d = nc.dram_tensor(
        "out", [SPC, COUT, H * W_SP], f16, kind="ExternalOutput"
    ).ap()

    with tile.TileContext(nc) as tc:
        with (
            tc.tile_pool(name="const", bufs=1) as cst,
            tc.tile_pool(name="x", bufs=1) as xpool,
            tc.tile_pool(name="wmix", bufs=1) as wmp,
            tc.tile_pool(name="ob", bufs=2) as opool,
            tc.tile_pool(name="ps", bufs=8, space="PSUM") as pspool,
        ):
            # --- HAM warm-up: dummy matmuls on a zeroed tile during loads
            zt = cst.tile([128, 512], f16, tag="zero")
            nc.gpsimd.memset(zt[:], 0.0)
            warm_ps = pspool.tile([128, 512], f32, tag="ps")
            for _ in range(N_WARM):
                nc.tensor.matmul(
                    warm_ps[:], zt[:, :128], zt[:], start=True, stop=True
                )
            for _ in range(N_WARM_SM):
                nc.tensor.matmul(
                    warm_ps[:, :128], zt[:, :128], zt[:, :128],
                    start=True, stop=True,
                )

            wt_t = cst.tile([CIN, RBW + NTAP * TAPW], f16, tag="wt")
            rb_t = wt_t[:, 0:RBW].bitcast(f32)  # [128, SPC*E] fp32

            def load_wt_chunk(t0, ntaps, eng):
                # first chunk also carries the routing scalars
                lo = 0 if t0 == 0 else RBW + t0 * TAPW
                sl = slice(lo, RBW + (t0 + ntaps) * TAPW)
                eng.dma_start(wt_t[:, sl], wt_d[:, sl])

            def load_x_chunk(s, xtiles, xch, c, eng):
                r0, nr = xch[c]
                xt = xpool.tile([CIN, nr * WP], f16, tag=f"x{s}_{c}",
                                name=f"x{s}_{c}")
                sl = slice(r0 * WP, (r0 + nr) * WP)
                eng.dma_start(xt[:], xpad_d[s][:, sl])
                xtiles[c] = xt

            # Two DGE rings only — concurrent queues share HBM bandwidth
            # round-robin, so extra rings would steal from the critical
            # prefix. Strict global need order, alternating rings: the
            # two first-matmul gates (tap-0 weights / x rows 0-9) lead
            # each ring, then x0 and the per-tap weight chunks interleave
            # by need time, sample-1's x strictly last. Stores later
            # reuse the sync ring.
            x0t = [None] * len(XCH0)
            x1t = [None] * len(XCH1)
            load_wt_chunk(0, 1, nc.sync)           # rb + tap 0 weights
            load_x_chunk(0, x0t, XCH0, 0, nc.scalar)
            load_x_chunk(0, x0t, XCH0, 1, nc.sync)
            load_wt_chunk(1, 1, nc.scalar)
            load_x_chunk(0, x0t, XCH0, 2, nc.sync)
            load_x_chunk(0, x0t, XCH0, 3, nc.scalar)
            load_x_chunk(0, x0t, XCH0, 4, nc.sync)
            load_wt_chunk(2, 1, nc.scalar)
            load_wt_chunk(3, 1, nc.sync)
            load_wt_chunk(4, 2, nc.scalar)
            load_wt_chunk(6, 3, nc.sync)
            load_x_chunk(1, x1t, XCH1, 0, nc.scalar)
            load_x_chunk(1, x1t, XCH1, 1, nc.sync)

            # warm the Activation engine's Copy table during the load
            # phase so the first real PSUM drain doesn't pay the load
            aw = cst.tile([128, 1], f16, tag="actwarm")
            nc.scalar.copy(aw[:], zt[:, 0:1])

            wt3 = wt_t[:, RBW:].rearrange("p (t e c) -> p t e c", t=NTAP, e=E)

            def mix(dst3, s, t0, t1, e_lo=0, e_hi=E):
                """dst3 = sum_{e in [e_lo,e_hi)} rb[s,e] * wt[:, t0:t1, e, :]"""
                first = True
                for e in range(e_lo, e_hi):
                    sc = rb_t[:, s * E + e : s * E + e + 1]
                    src = wt3[:, t0:t1, e, :]
                    if first:
                        nc.vector.tensor_scalar_mul(dst3, src, sc)
                        first = False
                    else:
                        nc.vector.scalar_tensor_tensor(
                            dst3, src, sc, dst3, MUL, ADD
                        )

            # tap 0 as two 2-expert half-mixes: the first matmul only
            # waits a 2-op mix chain instead of 4, and the second half
            # rides the PSUM accumulation as an extra matmul pass (free
            # during the x-load chase)
            wm0a = wmp.tile([CIN, COUT], f16, tag="wm0a")
            wm0b = wmp.tile([CIN, COUT], f16, tag="wm0b")
            mix(wm0a.rearrange("p (t c) -> p t c", t=1), 0, 0, 1, 0, 2)
            mix(wm0b.rearrange("p (t c) -> p t c", t=1), 0, 0, 1, 2, 4)

            def mix_chunks(s, chlist, prefix):
                out = {}
                for c, (t0, ntc) in enumerate(chlist):
                    wmt = wmp.tile(
                        [CIN, ntc * COUT], f16, tag=f"{prefix}{c}",
                        name=f"{prefix}{c}",
                    )
                    wm3 = wmt.rearrange("p (t c) -> p t c", t=ntc)
                    mix(wm3, s, t0, t0 + ntc)
                    for tt in range(t0, t0 + ntc):
                        out[tt] = (wmt, tt - t0)
                return out

            wm0 = mix_chunks(0, MIXCH, "wm0_")
            wm1 = mix_chunks(1, MIXCH1, "wm1_")

            def rhs_ap(xtiles, c, r0, nr, kh, kw):
                xch = XCH0 if xtiles is x0t else XCH1
                loc = r0 - xch[c][0]
                x3 = xtiles[c][:].rearrange("p (h w) -> p h w", w=WP)
                return x3[:, loc + kh : loc + kh + nr, kw : kw + W_SP]

            def copy_block(eng, ob, ps, r0, nr):
                sl = slice(r0 * W_SP, (r0 + nr) * W_SP)
                if eng is nc.scalar:
                    nc.scalar.copy(ob[:, sl], ps[:])
                else:
                    eng.tensor_copy(ob[:, sl], ps[:])

            # ---- sample 0: tap-outer over 7 live PSUM banks
            ps_map = {}
            for blk in range(NBLK):
                ps_map[blk] = pspool.tile(
                    [COUT, NT], f32, tag="ps", name=f"ps0_{blk}"
                )

            def pad_pe(n):
                for _ in range(n):
                    nc.tensor.matmul(
                        warm_ps[:, :128], zt[:, :128], zt[:, :128],
                        start=True, stop=True,
                    )

            passes = [(0, wm0a, 0, True, False), (0, wm0b, 0, False, False)]
            for t in range(1, NTAP):
                chunk, loc = wm0[t]
                passes.append((t, chunk, loc, False, t == NTAP - 1))

            for pi, (t, chunk, loc, start, stop) in enumerate(passes):
                kh, kw = divmod(t, KW)
                for blk in range(NBLK):
                    nc.tensor.matmul(
                        ps_map[blk][:],
                        chunk[:, loc * COUT : (loc + 1) * COUT],
                        rhs_ap(x0t, BLK_CH0[blk], blk * RPB, RPB, kh, kw),
                        start=start,
                        stop=stop,
                        skip_group_check=True,
                    )
                    pad_pe(PADS.get((pi, blk), 0))

            # drain sample 0: fp32 PSUM -> fp16 SBUF on Scalar/Vector in
            # parallel, then one large-line store for the whole sample
            ob0 = opool.tile([COUT, H * W_SP], f16, tag="ob")
            for blk in range(NBLK):
                eng = nc.scalar if blk % 2 == 0 else nc.vector
                copy_block(eng, ob0, ps_map[blk], blk * RPB, RPB)
            nc.sync.dma_start(out_d[0], ob0[:])

            # ---- sample 1: block-outer, drains incrementally with
            # batched stores (blocks 0-3, 4-5, 6, 7)
            ob1 = opool.tile([COUT, H * W_SP], f16, tag="ob")
            store_after = {3: slice(0, 32 * W_SP),
                           5: slice(32 * W_SP, 48 * W_SP),
                           6: slice(48 * W_SP, 54 * W_SP),
                           7: slice(54 * W_SP, 56 * W_SP)}
            for blk, (r0, nr, c) in enumerate(BLKS1):
                ps = pspool.tile(
                    [COUT, nr * W_SP], f32, tag="ps", name=f"ps1_{blk}"
                )
                for t in range(NTAP):
                    kh, kw = divmod(t, KW)
                    chunk, loc = wm1[t]
                    nc.tensor.matmul(
                        ps[:],
                        chunk[:, loc * COUT : (loc + 1) * COUT],
                        rhs_ap(x1t, c, r0, nr, kh, kw),
                        start=(t == 0),
                        stop=(t == NTAP - 1),
                    )
                # last block's copy AND store both on scalar: same-engine
                # ordering avoids a cross-engine semaphore hop in the tail
                last = blk == len(BLKS1) - 1
                eng = nc.scalar if (blk % 2 == 0 or last) else nc.vector
                copy_block(eng, ob1, ps, r0, nr)
                if blk in store_after:
                    # last store goes out on the idle Scalar ring so its
                    # descriptor generation isn't queued behind the
                    # previous store on sync (shorter kernel tail)
                    sl = store_after[blk]
                    seng = nc.scalar if blk == len(BLKS1) - 1 else nc.sync
                    seng.dma_start(out_d[1][:, sl], ob1[:, sl])

    nc.compile()
    return nc


def _get_nc():
    global _cached_nc
    if _cached_nc is None:
        _cached_nc = _build_nc()
    return _cached_nc


def _prep_inputs(x, routing_weights, W):
    x = np.ascontiguousarray(x, dtype=np.float32)
    routing_weights = np.ascontiguousarray(routing_weights, dtype=np.float32)
    W = np.ascontiguousarray(W, dtype=np.float32)

    xpad = np.zeros((B, CIN, HP, WP), np.float16)
    xpad[:, :, 1 : H + 1, 1 : W_SP + 1] = x.reshape(B, CIN, H, W_SP)
    xpad = xpad.reshape(B, CIN, HP * WP)

    # W[e, co, ci, kh, kw] -> wt[ci, (kh, kw, e, co)], with the per-core
    # routing scalars (broadcast over partitions) prepended
    wt = np.ascontiguousarray(
        np.transpose(W, (2, 3, 4, 0, 1)).astype(np.float16)
    ).reshape(CIN, NTAP * E * COUT)

    in_maps = []
    for c in range(NCORES):
        r = routing_weights[c * SPC : (c + 1) * SPC]  # fp32 [SPC, E]
        rb16 = r.reshape(1, SPC * E).view(np.float16)  # fp32 bits as fp16 pairs
        rb = np.broadcast_to(rb16, (128, SPC * E * 2))
        in_maps.append(
            {
                "xpad": xpad[c * SPC : (c + 1) * SPC],
                "wt": np.ascontiguousarray(np.concatenate([rb, wt], axis=1)),
            }
        )
    return in_maps


def _run(in_maps, **kwargs):
    from concourse import bass_utils

    nc = _get_nc()
    res = bass_utils.run_bass_kernel_spmd(
        nc, in_maps, core_ids=list(range(NCORES)), **kwargs
    )
    out = np.concatenate(
        [res.results[c]["out"].astype(np.float32) for c in range(NCORES)],
        axis=0,
    ).reshape(B, COUT, H, W_SP)
    return out, res


def kernel(x, routing_weights, W):
    in_maps = _prep_inputs(x, routing_weights, W)
    out, _ = _run(in_maps)
    return out
